# revision 3
# baseline (speedup 1.0000x reference)
"""Adaptive softmax (head + 2 factorized tails) on 8 TRN2 NeuronCores.

Strategy: pure data-parallel over the 4096 tokens (512/core, weights
replicated). Host-side prep: weights pre-transposed to [K, V] layout and
cast to bf16 so the TensorEngine consumes them directly; head bias folded
in as an extra contraction row against a ones-row appended to x.

Per core, per cluster: TensorE streams weight chunks into PSUM logits;
DVE evacuates PSUM into resident bf16 logit tiles; ACT computes
Exp(PSUM) into a throwaway scratch with accum_out giving the row-sum Z
partials; the output pass is a per-row tensor_scalar subtract
out = logit - (lse_tail + lse_head - head_cluster_logit)
on GpSimd (tails) / DVE (head), staged to f32 and DMA'd out.
"""

import sys
import types

for _p in ("/opt/trn_rl_repo",):
    if _p not in sys.path:
        sys.path.append(_p)

import numpy as np
import ml_dtypes

N, H = 4096, 1024
CUT0, CUT1, VOCAB = 4000, 20000, 50257
HEAD_OUT = CUT0 + 2            # 4002
HEAD_PAD = 4096                # padded head cols (pad logit = -30 via bias row)
P0, P1 = 1024, 256
NCORES = 8
T = N // NCORES                # 512 tokens per core
TT = T // 128                  # 4 token tiles
KX = 9                         # x k-tiles: 8 real + 1 (ones row for head bias)
KAUG = KX * 128                # 1152

BF16 = ml_dtypes.bfloat16

_COMPILED = {}


def _ceil_div(a, b):
    return -(-a // b)


def _chunks(total, width):
    return [(s, min(width, total - s)) for s in range(0, total, width)]


def _build():
    import concourse.tile as tile
    from concourse import bacc, mybir

    F32 = mybir.dt.float32
    BF = mybir.dt.bfloat16
    Exp = mybir.ActivationFunctionType.Exp
    Ln = mybir.ActivationFunctionType.Ln
    AX = mybir.AxisListType.X

    nc = bacc.Bacc("TRN2", target_bir_lowering=False, debug=False,
                   num_devices=NCORES)

    xT_d = nc.dram_tensor("xT", [KAUG, T], BF, kind="ExternalInput").ap()
    hwT_d = nc.dram_tensor("hwT", [KAUG, HEAD_PAD], BF, kind="ExternalInput").ap()
    w01_d = nc.dram_tensor("w01T", [H, P0], BF, kind="ExternalInput").ap()
    w02_d = nc.dram_tensor("w02T", [P0, CUT1 - CUT0], BF, kind="ExternalInput").ap()
    w11_d = nc.dram_tensor("w11T", [H, P1], BF, kind="ExternalInput").ap()
    w12_d = nc.dram_tensor("w12T", [P1, VOCAB - CUT1], BF, kind="ExternalInput").ap()
    out_d = nc.dram_tensor("out", [T, VOCAB], F32, kind="ExternalOutput").ap()

    x_r = xT_d.rearrange("(k p) t -> p k t", p=128)        # [128, 9, 512]
    hw_r = hwT_d.rearrange("(k p) v -> p k v", p=128)      # [128, 9, 4096]
    w01_r = w01_d.rearrange("(k p) m -> p k m", p=128)     # [128, 8, 1024]
    w02_r = w02_d.rearrange("(k p) v -> p k v", p=128)     # [128, 8, 16000]
    w11_r = w11_d.rearrange("(k p) m -> p k m", p=128)     # [128, 8, 256]
    w12_r = w12_d.rearrange("(k p) v -> p k v", p=128)     # [128, 2, 30257]

    V0 = CUT1 - CUT0            # 16000
    V1 = VOCAB - CUT1           # 30257

    with tile.TileContext(nc, pool_alloc_mode="queue") as tc:
        with (
            tc.tile_pool(name="persist", bufs=1) as persist,
            tc.tile_pool(name="smalls", bufs=1) as smalls,
            tc.tile_pool(name="stage", bufs=3) as stage,
            tc.tile_pool(name="scratch", bufs=3) as scratch,
            tc.tile_pool(name="psum", bufs=4, space="PSUM") as psum_pool,
        ):
            # ---- persistent activations ----
            xT_s = persist.tile([128, KX, T], BF, tag="xT")
            nc.sync.dma_start(out=xT_s, in_=x_r)
            h0T_s = persist.tile([128, 8, T], BF, tag="h0T")
            h1T_s = persist.tile([128, 2, T], BF, tag="h1T")

            # ---- tiny per-row scalars ----
            zb = smalls.tile([128, 1], F32, tag="zb")       # zero bias for ACT
            nc.vector.memset(zb, 0.0)

            def sc(tag):
                return smalls.tile([128, 1], F32, tag=tag, name=tag)

            lse_h = [sc(f"lse_h{t}") for t in range(TT)]
            l40 = [sc(f"l40_{t}") for t in range(TT)]       # head logit col 4000
            l41 = [sc(f"l41_{t}") for t in range(TT)]       # head logit col 4001
            d0 = [sc(f"d0_{t}") for t in range(TT)]
            d1 = [sc(f"d1_{t}") for t in range(TT)]
            Ztmp = [sc(f"Ztmp{t}") for t in range(TT)]
            lse_t = [sc(f"lse_t{t}") for t in range(TT)]
            zh_p = [smalls.tile([128, 4], F32, tag=f"zh_p{t}", name=f"zh_p{t}")
                    for t in range(TT)]
            z0_p = [smalls.tile([128, 16], F32, tag=f"z0_p{t}", name=f"z0_p{t}")
                    for t in range(TT)]
            z1_p = [smalls.tile([128, 32], F32, tag=f"z1_p{t}", name=f"z1_p{t}")
                    for t in range(TT)]

            # ---- phase 0: h0T = w01T.T @ xT, h1T = w11T.T @ xT (bf16) ----
            with tc.tile_pool(name="w1tmp", bufs=1) as w1tmp:
                w01_s = w1tmp.tile([128, 8, P0], BF, tag="w01")
                nc.sync.dma_start(out=w01_s, in_=w01_r)
                w11_s = w1tmp.tile([128, 8, P1], BF, tag="w11")
                nc.sync.dma_start(out=w11_s, in_=w11_r)

                for mpair in range(4):   # h0T: 8 m-tiles, 2 per psum tile
                    ps = psum_pool.tile([128, 1024], F32, tag="ps", name="ps_h0")
                    for half in range(2):
                        m = 2 * mpair + half
                        for k in range(8):
                            nc.tensor.matmul(
                                ps[:, half * 512:(half + 1) * 512],
                                lhsT=w01_s[:, k, m * 128:(m + 1) * 128],
                                rhs=xT_s[:, k, :],
                                start=(k == 0), stop=(k == 7),
                            )
                    nc.vector.tensor_copy(
                        out=h0T_s[:, 2 * mpair, :], in_=ps[:, 0:512])
                    nc.vector.tensor_copy(
                        out=h0T_s[:, 2 * mpair + 1, :], in_=ps[:, 512:1024])

                ps1 = psum_pool.tile([128, 1024], F32, tag="ps", name="ps_h1")
                for m in range(2):
                    for k in range(8):
                        nc.tensor.matmul(
                            ps1[:, m * 512:(m + 1) * 512],
                            lhsT=w11_s[:, k, m * 128:(m + 1) * 128],
                            rhs=xT_s[:, k, :],
                            start=(k == 0), stop=(k == 7),
                        )
                nc.vector.tensor_copy(out=h1T_s[:, 0, :], in_=ps1[:, 0:512])
                nc.vector.tensor_copy(out=h1T_s[:, 1, :], in_=ps1[:, 512:1024])

            # ---- generic cluster processor ----
            def run_cluster(name, wT_r, Vtot, Kt, lhsT_of, tts, logit_tiles,
                            zp, load_w, wpool, wbufs):
                loads = _chunks(Vtot, load_w)
                supers = _chunks(Vtot, 1024)
                ld_tiles = {}

                def load(li):
                    c0, w = loads[li]
                    t_ = wpool.tile([128, Kt, load_w], BF, tag=f"w_{name}",
                                    bufs=wbufs, name=f"w_{name}")
                    nc.sync.dma_start(out=t_[:, :, :w], in_=wT_r[:, :, c0:c0 + w])
                    return t_

                for si, (sc0, sw) in enumerate(supers):
                    li0 = sc0 // load_w
                    li1 = _ceil_div(sc0 + sw, load_w)
                    for li in range(li0, li1):
                        if li not in ld_tiles:
                            ld_tiles[li] = load(li)
                    for tt in tts:
                        ps = psum_pool.tile([128, 1024], F32, tag="ps",
                                            name=f"ps_{name}")
                        for (c0, cw) in _chunks(sw, 512):
                            vc0 = sc0 + c0
                            li = vc0 // load_w
                            off = vc0 - loads[li][0]
                            wt = ld_tiles[li]
                            for k in range(Kt):
                                nc.tensor.matmul(
                                    ps[:, c0:c0 + cw],
                                    lhsT=lhsT_of(k, tt),
                                    rhs=wt[:, k, off:off + cw],
                                    start=(k == 0), stop=(k == Kt - 1),
                                )
                        # evacuate logits (bf16) on DVE
                        nc.vector.tensor_copy(
                            out=logit_tiles[tt][:, sc0:sc0 + sw],
                            in_=ps[:, :sw])
                        # exp + row-sum partial on ACT (output discarded)
                        ex = scratch.tile([128, 1024], BF, tag="ex", name="ex")
                        nc.scalar.activation(
                            out=ex[:, :sw], in_=ps[:, :sw],
                            func=Exp, bias=zb, scale=1.0,
                            accum_out=zp[tt][:, si:si + 1],
                        )
                    for li in range(li0, li1):
                        if (li + 1) * load_w <= sc0 + sw:
                            ld_tiles.pop(li, None)

            def finalize(eng, tt, logit_tile, d_ap, out_c0, out_w):
                """out[tt rows, out_c0:+out_w] = logit - d  (per-row scalar)."""
                r0 = tt * 128
                for (c0, cw) in _chunks(out_w, 2048):
                    st = stage.tile([128, 2048], F32, tag="st", name="st")
                    eng.tensor_scalar_sub(st[:, :cw],
                                          logit_tile[:, c0:c0 + cw], d_ap)
                    nc.sync.dma_start(
                        out=out_d[r0:r0 + 128, out_c0 + c0:out_c0 + c0 + cw],
                        in_=st[:, :cw],
                    )

            # ---- phase 1: head ----
            with tc.tile_pool(name="headph", bufs=1) as headph:
                lgh = [headph.tile([128, HEAD_PAD], BF, tag=f"lgh{t}",
                                   name=f"lgh{t}") for t in range(TT)]
                run_cluster(
                    "h", hw_r, HEAD_PAD, KX,
                    lambda k, tt: xT_s[:, k, tt * 128:(tt + 1) * 128],
                    list(range(TT)), lgh, zh_p, 512, headph, 3,
                )
                for tt in range(TT):
                    nc.vector.reduce_sum(out=Ztmp[tt], in_=zh_p[tt][:, 0:4],
                                         axis=AX)
                    nc.scalar.activation(out=lse_h[tt], in_=Ztmp[tt],
                                         func=Ln, bias=zb, scale=1.0)
                    nc.vector.tensor_copy(out=l40[tt],
                                          in_=lgh[tt][:, CUT0:CUT0 + 1])
                    nc.vector.tensor_copy(out=l41[tt],
                                          in_=lgh[tt][:, CUT0 + 1:CUT0 + 2])
                    finalize(nc.vector, tt, lgh[tt], lse_h[tt], 0, CUT0)

            # ---- phase 2: tail 0 (16000 cols, K=8 over h0T) ----
            with tc.tile_pool(name="t0ph", bufs=1) as t0ph:
                lg0 = [t0ph.tile([128, V0], BF, tag=f"lg0_{t}",
                                 name=f"lg0_{t}") for t in range(TT)]
                run_cluster(
                    "t0", w02_r, V0, 8,
                    lambda k, tt: h0T_s[:, k, tt * 128:(tt + 1) * 128],
                    list(range(TT)), lg0, z0_p, 512, t0ph, 3,
                )
                for tt in range(TT):
                    nc.vector.reduce_sum(out=Ztmp[tt], in_=z0_p[tt][:, 0:16],
                                         axis=AX)
                    nc.scalar.activation(out=lse_t[tt], in_=Ztmp[tt],
                                         func=Ln, bias=zb, scale=1.0)
                    # d0 = lse_t0 + lse_h - l4000
                    nc.vector.tensor_add(d0[tt], lse_t[tt], lse_h[tt])
                    nc.vector.tensor_sub(d0[tt], d0[tt], l40[tt])
                    finalize(nc.gpsimd, tt, lg0[tt], d0[tt], CUT0, V0)

            # ---- phase 3: tail 1 (30257 cols, K=2 over h1T), 2 tt-groups ----
            with tc.tile_pool(name="t1ph", bufs=1) as t1ph:
                for grp in range(2):
                    tts = [2 * grp, 2 * grp + 1]
                    lg1 = {tt: t1ph.tile([128, V1], BF, tag="lg1",
                                         bufs=2, name=f"lg1_{tt}")
                           for tt in tts}
                    run_cluster(
                        "t1", w12_r, V1, 2,
                        lambda k, tt: h1T_s[:, k, tt * 128:(tt + 1) * 128],
                        tts, lg1, z1_p, 1024, t1ph, 2,
                    )
                    for tt in tts:
                        nc.vector.reduce_sum(out=Ztmp[tt],
                                             in_=z1_p[tt][:, 0:30], axis=AX)
                        nc.scalar.activation(out=lse_t[tt], in_=Ztmp[tt],
                                             func=Ln, bias=zb, scale=1.0)
                        nc.vector.tensor_add(d1[tt], lse_t[tt], lse_h[tt])
                        nc.vector.tensor_sub(d1[tt], d1[tt], l41[tt])
                        finalize(nc.gpsimd, tt, lg1[tt], d1[tt], CUT1, V1)

    nc.compile()
    return nc


def _get_nc():
    if "nc" not in _COMPILED:
        _COMPILED["nc"] = _build()
    return _COMPILED["nc"]


def _prep_inputs(x, head_w, head_b, t0_w1, t0_w2, t1_w1, t1_w2):
    f32 = np.float32

    hwT = np.zeros((KAUG, HEAD_PAD), dtype=f32)
    hwT[:H, :HEAD_OUT] = np.asarray(head_w, f32).T
    hwT[H, :HEAD_OUT] = np.asarray(head_b, f32)
    hwT[H, HEAD_OUT:] = -30.0
    hwT = hwT.astype(BF16)

    w01T = np.ascontiguousarray(np.asarray(t0_w1, f32).T).astype(BF16)
    w02T = np.ascontiguousarray(np.asarray(t0_w2, f32).T).astype(BF16)
    w11T = np.ascontiguousarray(np.asarray(t1_w1, f32).T).astype(BF16)
    w12T = np.ascontiguousarray(np.asarray(t1_w2, f32).T).astype(BF16)

    in_maps = []
    for c in range(NCORES):
        xs = np.asarray(x[c * T:(c + 1) * T], f32)
        xT = np.zeros((KAUG, T), dtype=f32)
        xT[:H] = xs.T
        xT[H] = 1.0
        in_maps.append({
            "xT": xT.astype(BF16),
            "hwT": hwT,
            "w01T": w01T,
            "w02T": w02T,
            "w11T": w11T,
            "w12T": w12T,
        })
    return in_maps


def run(trace=False, **inputs):
    from concourse.bass_utils import run_bass_kernel_spmd

    if trace:
        try:
            if "antenv.axon_hooks" not in sys.modules:
                if "/root/.axon_site" not in sys.path:
                    sys.path.append("/root/.axon_site")
                import trn_agent_boot.trn_boot as tb
                hook = tb._ntff_profile_via_ctypes("/opt/axon/libaxon_pjrt.so")
                mod = types.ModuleType("antenv.axon_hooks")
                mod.get_axon_ntff_profile_hook = lambda: hook
                sys.modules["antenv.axon_hooks"] = mod
        except Exception:
            trace = False

    nc = _get_nc()
    in_maps = _prep_inputs(**inputs)
    res = run_bass_kernel_spmd(nc, in_maps, core_ids=list(range(NCORES)),
                               trace=trace)
    out = np.concatenate([res.results[i]["out"] for i in range(NCORES)], axis=0)
    return out, res


def kernel(**inputs):
    out, _ = run(trace=False, **inputs)
    return out


if __name__ == "__main__":
    rng = np.random.default_rng(0)
    ins = {
        "x": rng.standard_normal((N, H), dtype=np.float32),
        "head_w": (rng.standard_normal((HEAD_OUT, H), dtype=np.float32) / 32),
        "head_b": (rng.standard_normal(HEAD_OUT).astype(np.float32) * 0.01),
        "t0_w1": (rng.standard_normal((P0, H), dtype=np.float32) / 32),
        "t0_w2": (rng.standard_normal((CUT1 - CUT0, P0), dtype=np.float32) / 32),
        "t1_w1": (rng.standard_normal((P1, H), dtype=np.float32) / 32),
        "t1_w2": (rng.standard_normal((VOCAB - CUT1, P1), dtype=np.float32) / 16),
    }
    out, res = run(trace=False, **ins)
    print("out", out.shape, out.dtype)


# revision 6
# speedup vs baseline: 3.7410x; 3.7410x over previous
"""Adaptive softmax (head + 2 factorized tails) on 8 TRN2 NeuronCores.

Strategy: pure data-parallel over the 4096 tokens (512/core, weights
replicated). Host-side prep: weights pre-transposed to [K, V] layout and
cast to bf16 so the TensorEngine consumes them directly; head bias folded
in as an extra contraction row against a ones-row appended to x.

Per core, per cluster: TensorE streams weight chunks into PSUM logits;
DVE evacuates PSUM into resident bf16 logit tiles; ACT computes
Exp(PSUM) into a throwaway scratch with accum_out giving the row-sum Z
partials; the output pass is a per-row tensor_scalar subtract
out = logit - (lse_tail + lse_head - head_cluster_logit)
on GpSimd (tails) / DVE (head), staged to f32 and DMA'd out.
"""

import sys
import types

for _p in ("/opt/trn_rl_repo",):
    if _p not in sys.path:
        sys.path.append(_p)

import numpy as np
import ml_dtypes

N, H = 4096, 1024
CUT0, CUT1, VOCAB = 4000, 20000, 50257
HEAD_OUT = CUT0 + 2            # 4002
HEAD_PAD = 4096                # padded head cols (pad logit = -30 via bias row)
P0, P1 = 1024, 256
NCORES = 8
T = N // NCORES                # 512 tokens per core
TT = T // 128                  # 4 token tiles
KX = 9                         # x k-tiles: 8 real + 1 (ones row for head bias)
KAUG = KX * 128                # 1152

BF16 = ml_dtypes.bfloat16

_COMPILED = {}


def _ceil_div(a, b):
    return -(-a // b)


def _chunks(total, width):
    return [(s, min(width, total - s)) for s in range(0, total, width)]


def _build():
    import concourse.tile as tile
    from concourse import bacc, mybir

    F32 = mybir.dt.float32
    BF = mybir.dt.bfloat16
    Exp = mybir.ActivationFunctionType.Exp
    Ln = mybir.ActivationFunctionType.Ln
    AX = mybir.AxisListType.X

    nc = bacc.Bacc("TRN2", target_bir_lowering=False, debug=False,
                   num_devices=NCORES)

    xT_d = nc.dram_tensor("xT", [KAUG, T], BF, kind="ExternalInput").ap()
    hwT_d = nc.dram_tensor("hwT", [KAUG, HEAD_PAD], BF, kind="ExternalInput").ap()
    w01_d = nc.dram_tensor("w01T", [H, P0], BF, kind="ExternalInput").ap()
    w02_d = nc.dram_tensor("w02T", [P0, CUT1 - CUT0], BF, kind="ExternalInput").ap()
    w11_d = nc.dram_tensor("w11T", [H, P1], BF, kind="ExternalInput").ap()
    w12_d = nc.dram_tensor("w12T", [P1, VOCAB - CUT1], BF, kind="ExternalInput").ap()
    out_d = nc.dram_tensor("out", [T, VOCAB], F32, kind="ExternalOutput").ap()

    x_r = xT_d.rearrange("(k p) t -> p k t", p=128)        # [128, 9, 512]
    hw_r = hwT_d.rearrange("(k p) v -> p k v", p=128)      # [128, 9, 4096]
    w01_r = w01_d.rearrange("(k p) m -> p k m", p=128)     # [128, 8, 1024]
    w02_r = w02_d.rearrange("(k p) v -> p k v", p=128)     # [128, 8, 16000]
    w11_r = w11_d.rearrange("(k p) m -> p k m", p=128)     # [128, 8, 256]
    w12_r = w12_d.rearrange("(k p) v -> p k v", p=128)     # [128, 2, 30257]

    V0 = CUT1 - CUT0            # 16000
    V1 = VOCAB - CUT1           # 30257

    with tile.TileContext(nc, pool_alloc_mode="queue") as tc:
        with (
            tc.tile_pool(name="persist", bufs=1) as persist,
            tc.tile_pool(name="smalls", bufs=1) as smalls,
            tc.tile_pool(name="stage", bufs=3) as stage,
            tc.tile_pool(name="scratch", bufs=3) as scratch,
            tc.tile_pool(name="psum", bufs=4, space="PSUM") as psum_pool,
        ):
            # ---- persistent activations ----
            xT_s = persist.tile([128, KX, T], BF, tag="xT")
            nc.sync.dma_start(out=xT_s, in_=x_r)
            h0T_s = persist.tile([128, 8, T], BF, tag="h0T")
            h1T_s = persist.tile([128, 2, T], BF, tag="h1T")

            # ---- tiny per-row scalars ----
            zb = smalls.tile([128, 1], F32, tag="zb")       # zero bias for ACT
            nc.vector.memset(zb, 0.0)

            def sc(tag):
                return smalls.tile([128, 1], F32, tag=tag, name=tag)

            lse_h = [sc(f"lse_h{t}") for t in range(TT)]
            l40 = [sc(f"l40_{t}") for t in range(TT)]       # head logit col 4000
            l41 = [sc(f"l41_{t}") for t in range(TT)]       # head logit col 4001
            d0 = [sc(f"d0_{t}") for t in range(TT)]
            d1 = [sc(f"d1_{t}") for t in range(TT)]
            nd1 = [sc(f"nd1_{t}") for t in range(TT)]   # -d1 (ACT bias form)
            Ztmp = [sc(f"Ztmp{t}") for t in range(TT)]
            lse_t = [sc(f"lse_t{t}") for t in range(TT)]
            zh_p = [smalls.tile([128, 4], F32, tag=f"zh_p{t}", name=f"zh_p{t}")
                    for t in range(TT)]
            z0_p = [smalls.tile([128, 16], F32, tag=f"z0_p{t}", name=f"z0_p{t}")
                    for t in range(TT)]
            z1_p = [smalls.tile([128, 32], F32, tag=f"z1_p{t}", name=f"z1_p{t}")
                    for t in range(TT)]

            # ---- phase 0: h0T = w01T.T @ xT, h1T = w11T.T @ xT (bf16) ----
            with tc.tile_pool(name="w1tmp", bufs=1) as w1tmp:
                w01_s = w1tmp.tile([128, 8, P0], BF, tag="w01")
                nc.sync.dma_start(out=w01_s, in_=w01_r)
                w11_s = w1tmp.tile([128, 8, P1], BF, tag="w11")
                nc.sync.dma_start(out=w11_s, in_=w11_r)

                for mpair in range(4):   # h0T: 8 m-tiles, 2 per psum tile
                    ps = psum_pool.tile([128, 1024], F32, tag="ps", name="ps_h0")
                    for half in range(2):
                        m = 2 * mpair + half
                        for k in range(8):
                            nc.tensor.matmul(
                                ps[:, half * 512:(half + 1) * 512],
                                lhsT=w01_s[:, k, m * 128:(m + 1) * 128],
                                rhs=xT_s[:, k, :],
                                start=(k == 0), stop=(k == 7),
                            )
                    nc.vector.tensor_copy(
                        out=h0T_s[:, 2 * mpair, :], in_=ps[:, 0:512])
                    nc.vector.tensor_copy(
                        out=h0T_s[:, 2 * mpair + 1, :], in_=ps[:, 512:1024])

                ps1 = psum_pool.tile([128, 1024], F32, tag="ps", name="ps_h1")
                for m in range(2):
                    for k in range(8):
                        nc.tensor.matmul(
                            ps1[:, m * 512:(m + 1) * 512],
                            lhsT=w11_s[:, k, m * 128:(m + 1) * 128],
                            rhs=xT_s[:, k, :],
                            start=(k == 0), stop=(k == 7),
                        )
                nc.vector.tensor_copy(out=h1T_s[:, 0, :], in_=ps1[:, 0:512])
                nc.vector.tensor_copy(out=h1T_s[:, 1, :], in_=ps1[:, 512:1024])

            # ---- generic cluster processor ----
            def run_cluster(name, wT_r, Vtot, Kt, lhsT_of, tts, logit_tiles,
                            zp, load_w, wpool, wbufs):
                loads = _chunks(Vtot, load_w)
                supers = _chunks(Vtot, 1024)
                ld_tiles = {}

                def load(li):
                    c0, w = loads[li]
                    t_ = wpool.tile([128, Kt, load_w], BF, tag=f"w_{name}",
                                    bufs=wbufs, name=f"w_{name}")
                    nc.sync.dma_start(out=t_[:, :, :w], in_=wT_r[:, :, c0:c0 + w])
                    return t_

                for si, (sc0, sw) in enumerate(supers):
                    li0 = sc0 // load_w
                    li1 = _ceil_div(sc0 + sw, load_w)
                    for li in range(li0, li1):
                        if li not in ld_tiles:
                            ld_tiles[li] = load(li)
                    for tt in tts:
                        ps = psum_pool.tile([128, 1024], F32, tag="ps",
                                            name=f"ps_{name}")
                        for (c0, cw) in _chunks(sw, 512):
                            vc0 = sc0 + c0
                            li = vc0 // load_w
                            off = vc0 - loads[li][0]
                            wt = ld_tiles[li]
                            for k in range(Kt):
                                nc.tensor.matmul(
                                    ps[:, c0:c0 + cw],
                                    lhsT=lhsT_of(k, tt),
                                    rhs=wt[:, k, off:off + cw],
                                    start=(k == 0), stop=(k == Kt - 1),
                                )
                        # evacuate logits (bf16) on DVE
                        nc.vector.tensor_copy(
                            out=logit_tiles[tt][:, sc0:sc0 + sw],
                            in_=ps[:, :sw])
                        # exp + row-sum partial on ACT (output discarded)
                        ex = scratch.tile([128, 1024], BF, tag="ex", name="ex")
                        nc.scalar.activation(
                            out=ex[:, :sw], in_=ps[:, :sw],
                            func=Exp, bias=zb, scale=1.0,
                            accum_out=zp[tt][:, si:si + 1],
                        )
                    for li in range(li0, li1):
                        if (li + 1) * load_w <= sc0 + sw:
                            ld_tiles.pop(li, None)

            def finalize(eng, tt, logit_tile, d_ap, out_c0, out_w):
                """out[tt rows, out_c0:+out_w] = logit - d  (per-row scalar).
                eng="dve": DVE tensor_scalar_sub with d_ap = d.
                eng="act": ACT Identity(in + bias) with d_ap = -d."""
                r0 = tt * 128
                for (c0, cw) in _chunks(out_w, 2048):
                    st = stage.tile([128, 2048], F32, tag="st", name="st")
                    if eng == "dve":
                        nc.vector.tensor_scalar_sub(
                            st[:, :cw], logit_tile[:, c0:c0 + cw], d_ap)
                    else:
                        nc.scalar.add(st[:, :cw], logit_tile[:, c0:c0 + cw],
                                      d_ap)
                    nc.sync.dma_start(
                        out=out_d[r0:r0 + 128, out_c0 + c0:out_c0 + c0 + cw],
                        in_=st[:, :cw],
                    )

            # ---- phase 1: head ----
            with tc.tile_pool(name="headph", bufs=1) as headph:
                lgh = [headph.tile([128, HEAD_PAD], BF, tag=f"lgh{t}",
                                   name=f"lgh{t}") for t in range(TT)]
                run_cluster(
                    "h", hw_r, HEAD_PAD, KX,
                    lambda k, tt: xT_s[:, k, tt * 128:(tt + 1) * 128],
                    list(range(TT)), lgh, zh_p, 512, headph, 3,
                )
                for tt in range(TT):
                    nc.vector.reduce_sum(out=Ztmp[tt], in_=zh_p[tt][:, 0:4],
                                         axis=AX)
                    nc.scalar.activation(out=lse_h[tt], in_=Ztmp[tt],
                                         func=Ln, bias=zb, scale=1.0)
                    nc.vector.tensor_copy(out=l40[tt],
                                          in_=lgh[tt][:, CUT0:CUT0 + 1])
                    nc.vector.tensor_copy(out=l41[tt],
                                          in_=lgh[tt][:, CUT0 + 1:CUT0 + 2])
                    finalize("dve", tt, lgh[tt], lse_h[tt], 0, CUT0)

            # ---- phase 2: tail 0 (16000 cols, K=8 over h0T) ----
            with tc.tile_pool(name="t0ph", bufs=1) as t0ph:
                lg0 = [t0ph.tile([128, V0], BF, tag=f"lg0_{t}",
                                 name=f"lg0_{t}") for t in range(TT)]
                run_cluster(
                    "t0", w02_r, V0, 8,
                    lambda k, tt: h0T_s[:, k, tt * 128:(tt + 1) * 128],
                    list(range(TT)), lg0, z0_p, 512, t0ph, 3,
                )
                for tt in range(TT):
                    nc.vector.reduce_sum(out=Ztmp[tt], in_=z0_p[tt][:, 0:16],
                                         axis=AX)
                    nc.scalar.activation(out=lse_t[tt], in_=Ztmp[tt],
                                         func=Ln, bias=zb, scale=1.0)
                    # d0 = lse_t0 + lse_h - l4000
                    nc.vector.tensor_add(d0[tt], lse_t[tt], lse_h[tt])
                    nc.vector.tensor_sub(d0[tt], d0[tt], l40[tt])
                    finalize("dve", tt, lg0[tt], d0[tt], CUT0, V0)

            # ---- phase 3: tail 1 (30257 cols, K=2 over h1T), 2 tt-groups ----
            with tc.tile_pool(name="t1ph", bufs=1) as t1ph:
                for grp in range(2):
                    tts = [2 * grp, 2 * grp + 1]
                    lg1 = {tt: t1ph.tile([128, V1], BF, tag="lg1",
                                         bufs=2, name=f"lg1_{tt}")
                           for tt in tts}
                    run_cluster(
                        "t1", w12_r, V1, 2,
                        lambda k, tt: h1T_s[:, k, tt * 128:(tt + 1) * 128],
                        tts, lg1, z1_p, 1024, t1ph, 2,
                    )
                    for tt in tts:
                        nc.vector.reduce_sum(out=Ztmp[tt],
                                             in_=z1_p[tt][:, 0:30], axis=AX)
                        nc.scalar.activation(out=lse_t[tt], in_=Ztmp[tt],
                                             func=Ln, bias=zb, scale=1.0)
                        nc.vector.tensor_add(d1[tt], lse_t[tt], lse_h[tt])
                        nc.vector.tensor_sub(d1[tt], d1[tt], l41[tt])
                        nc.vector.tensor_sub(nd1[tt], zb, d1[tt])
                        finalize("act", tt, lg1[tt], nd1[tt], CUT1, V1)

    nc.compile()
    return nc


def _get_nc():
    if "nc" not in _COMPILED:
        _COMPILED["nc"] = _build()
    return _COMPILED["nc"]


def _prep_inputs(x, head_w, head_b, t0_w1, t0_w2, t1_w1, t1_w2):
    f32 = np.float32

    hwT = np.zeros((KAUG, HEAD_PAD), dtype=f32)
    hwT[:H, :HEAD_OUT] = np.asarray(head_w, f32).T
    hwT[H, :HEAD_OUT] = np.asarray(head_b, f32)
    hwT[H, HEAD_OUT:] = -30.0
    hwT = hwT.astype(BF16)

    w01T = np.ascontiguousarray(np.asarray(t0_w1, f32).T).astype(BF16)
    w02T = np.ascontiguousarray(np.asarray(t0_w2, f32).T).astype(BF16)
    w11T = np.ascontiguousarray(np.asarray(t1_w1, f32).T).astype(BF16)
    w12T = np.ascontiguousarray(np.asarray(t1_w2, f32).T).astype(BF16)

    in_maps = []
    for c in range(NCORES):
        xs = np.asarray(x[c * T:(c + 1) * T], f32)
        xT = np.zeros((KAUG, T), dtype=f32)
        xT[:H] = xs.T
        xT[H] = 1.0
        in_maps.append({
            "xT": xT.astype(BF16),
            "hwT": hwT,
            "w01T": w01T,
            "w02T": w02T,
            "w11T": w11T,
            "w12T": w12T,
        })
    return in_maps


def run(trace=False, **inputs):
    from concourse.bass_utils import run_bass_kernel_spmd

    if trace:
        try:
            if "antenv.axon_hooks" not in sys.modules:
                if "/root/.axon_site" not in sys.path:
                    sys.path.append("/root/.axon_site")
                import trn_agent_boot.trn_boot as tb
                hook = tb._ntff_profile_via_ctypes("/opt/axon/libaxon_pjrt.so")
                mod = types.ModuleType("antenv.axon_hooks")
                mod.get_axon_ntff_profile_hook = lambda: hook
                sys.modules["antenv.axon_hooks"] = mod
        except Exception:
            trace = False

    nc = _get_nc()
    in_maps = _prep_inputs(**inputs)
    res = run_bass_kernel_spmd(nc, in_maps, core_ids=list(range(NCORES)),
                               trace=trace)
    out = np.concatenate([res.results[i]["out"] for i in range(NCORES)], axis=0)
    return out, res


def kernel(**inputs):
    out, _ = run(trace=False, **inputs)
    return out


if __name__ == "__main__":
    rng = np.random.default_rng(0)
    ins = {
        "x": rng.standard_normal((N, H), dtype=np.float32),
        "head_w": (rng.standard_normal((HEAD_OUT, H), dtype=np.float32) / 32),
        "head_b": (rng.standard_normal(HEAD_OUT).astype(np.float32) * 0.01),
        "t0_w1": (rng.standard_normal((P0, H), dtype=np.float32) / 32),
        "t0_w2": (rng.standard_normal((CUT1 - CUT0, P0), dtype=np.float32) / 32),
        "t1_w1": (rng.standard_normal((P1, H), dtype=np.float32) / 32),
        "t1_w2": (rng.standard_normal((VOCAB - CUT1, P1), dtype=np.float32) / 16),
    }
    out, res = run(trace=False, **ins)
    print("out", out.shape, out.dtype)
